# revision 1
# baseline (speedup 1.0000x reference)
"""Trainium2 Bass kernel for nn_EternalRecursion (GRUCell self-recursion, B=512, D=500).

Strategy
--------
Data-parallel over 8 NeuronCores: 64 batch rows per core, GRU weights replicated.

Math restructuring (host-side, exact):
  - After step 1 the reference feeds h_new as BOTH x and h of the GRU cell, so
    steps >= 2 use combined weights W_rz = (W_ih+W_hh)[0:1000] for the r/z gates,
    while the n-gate keeps W_ih_n / W_hh_n separate (r multiplies only the h-side).
  - Step 1 (x=state, h=0) uses W_ih with a zero block for the h-side n columns,
    which makes it the *same* device code path with different weights.
  - Biases are folded into the matmul via an extra contraction row of ones.
  - The break check "mean(h_k) > bc" latches the output at the first step k*
    whose global mean exceeds bc. The device free-runs L steps, records per-step
    per-partition sums (free side-output of the last fused DVE op), and the host
    computes the global means. If the break fires before the last step (it cannot
    for the harness inputs: |h|<1 and bc>=0.9 keeps means far below bc), the
    kernel is re-built with L=k* and re-run, which reproduces the latched output.

Device layout (per core, per step):
  - h is stored "packed": [128 partitions, 250 free] with partition 64*H+b
    holding h[b, 250*H + c]. All elementwise gate math runs on [128, 250] tiles.
  - Gate pre-activations are computed as 16 f32r matmuls with M=128 into two
    [128, 512] PSUM tiles: region 1 = [gr | gz] (sigmoids + the z-path overlap
    region 2's matmuls), region 2 = [gin | ghn]. Since f32r matmuls require
    dst partition base 0, both packed halves are produced by ONE matmul via a
    doubled contraction: the stationary holds h^T twice along K (gate-half G0
    channels with batch in array cols 0:64 + zeros, G1 channels in cols
    64:128), and the moving W rows carry the matching half's gate columns.
    8 K-tile groups x 2 regions of N=500.
  - The stationary h^T lives in one [126, 1024] SBUF tile; groups are ordered
    (D0,G0),(D1,G0),(D0,G1),(D1,G1),(D2,G0),(D3,G0),(D2,G1),(D3,G1) so the
    first four matmuls depend only on PE-transpose A of h_new, whose two
    PSUM->SBUF copies run in parallel on ACT and DVE. Ones rows in groups 1/3
    implement the bias fold; the complementary halves stay zero. The gate
    chain is split in two 125-column chunks so transpose A starts while chunk
    1 is still in the vector/scalar pipes.
"""

import os
import sys
import types
import numpy as np

D = 500
B = 512
NCORES = 8
BS = B // NCORES          # 64 batch rows per core
HALF = 250                # free columns of the packed layout
# K permutation: hT column-groups are [0:125 | 250:375 | 125:250 | 375:500]
PERM = np.concatenate([
    np.arange(0, 125), np.arange(250, 375),
    np.arange(125, 250), np.arange(375, 500),
])
# gate rows for the rz column blocks: [r 0:250 | z 0:250 | r 250:500 | z 250:500]
GATE_ROWS_RZ = np.concatenate([
    np.arange(0, 250), np.arange(500, 750),
    np.arange(250, 500), np.arange(750, 1000),
])


def _install_hook_module():
    """Provide antenv.axon_hooks (missing from the RO image) so NTFF tracing
    through bass_utils can work when requested. Harmless if anything fails."""
    if "antenv.axon_hooks" in sys.modules:
        return
    mod = types.ModuleType("antenv.axon_hooks")
    holder = [None]
    mod.set_axon_ntff_profile_hook = lambda h: holder.__setitem__(0, h)
    mod.get_axon_ntff_profile_hook = lambda: holder[0]
    sys.modules["antenv.axon_hooks"] = mod
    try:
        from trn_agent_boot.trn_boot import _ntff_profile_via_ctypes
        hook = _ntff_profile_via_ctypes("/opt/axon/libaxon_pjrt.so")
        mod.set_axon_ntff_profile_hook(hook)
    except Exception:
        pass


_install_hook_module()

import concourse.bass as bass  # noqa: E402
import concourse.mybir as mybir  # noqa: E402
import concourse.tile as tile  # noqa: E402
from concourse import bass_utils  # noqa: E402
from concourse.masks import make_identity  # noqa: E402
import bass_rust  # noqa: E402

F32 = mybir.dt.float32
F32R = mybir.dt.float32r
AF = mybir.ActivationFunctionType
ALU = mybir.AluOpType


def _split_overwide_waits(nc, maxw=1):
    """walrus here rejects >1 sync wait per instruction; spread extras over
    preceding NoOp carriers. Most multi-wait instructions get same-engine
    carriers (order-preserving); the kernel-end drain (many loose-end waits)
    gets carriers round-robined across all engines so they resolve in
    parallel before the final barrier instead of serially on one engine."""
    n_new = 0
    all_engines = (mybir.EngineType.SP, mybir.EngineType.Activation,
                   mybir.EngineType.PE, mybir.EngineType.DVE,
                   mybir.EngineType.Pool)
    for fn in nc.m.functions:
        for bb in fn.blocks:
            out = []
            for inst in bb.instructions:
                si = inst.sync_info
                if si is not None and si.on_wait and len(si.on_wait) > maxw:
                    waits = list(si.on_wait)
                    chunks = [waits[i:i + maxw] for i in range(0, len(waits), maxw)]
                    spread = len(chunks) > 4  # only the big end-of-kernel drain
                    for j, ch in enumerate(chunks[:-1]):
                        eng = all_engines[j % len(all_engines)] if spread                             else inst.engine
                        nd = mybir.InstNoOp(
                            name=f"I-swx{n_new}", engine=eng,
                            bass_nofuse=True,
                            sync_info=bass_rust.SyncInfo(on_wait=ch, on_update=[]))
                        n_new += 1
                        nc.register_instruction(nd, overwrite=True)
                        out.append(nd)
                    inst.sync_info = bass_rust.SyncInfo(
                        on_wait=chunks[-1], on_update=list(si.on_update or []))
                out.append(inst)
            bb.instructions = out
    return n_new


def _build(L):
    """Build the Bass module for L GRU steps. Returns nc."""
    assert L >= 1
    nc = bass.Bass("TRN2", target_bir_lowering=False, debug=False)

    statet_d = nc.dram_tensor("statet", [126, 1024], F32R, kind="ExternalInput").ap()
    wa_d = nc.dram_tensor("wa", [2, 126, 4000], F32R, kind="ExternalInput").ap()
    wb_d = nc.dram_tensor("wb", [2, 126, 4000], F32R, kind="ExternalInput").ap()
    hout_d = nc.dram_tensor("hout", [128, HALF], F32R, kind="ExternalOutput").ap()
    sums_d = nc.dram_tensor("sums", [128, 2 * L], F32, kind="ExternalOutput").ap()

    with tile.TileContext(nc) as tc:
        import contextlib
        with contextlib.ExitStack() as ctx:
            consts = ctx.enter_context(tc.tile_pool(name="consts", bufs=1))
            wpool = ctx.enter_context(tc.tile_pool(name="weights", bufs=1))
            hpool = ctx.enter_context(tc.tile_pool(name="hstate", bufs=1))
            work = ctx.enter_context(tc.tile_pool(name="work", bufs=2))
            gpsum = ctx.enter_context(tc.tile_pool(name="gpsum", bufs=2, space="PSUM"))
            tpsum = ctx.enter_context(tc.tile_pool(name="tpsum", bufs=2, space="PSUM"))

            identity = consts.tile([128, 128], F32, tag="identity", name="identity")
            make_identity(nc, identity[:])
            # f32r view for the transposes (verifier wants an f32r producer)
            identity_r = consts.tile([128, 128], F32R, tag="identity_r", name="identity_r")
            nc.vector.tensor_copy(identity_r[:], identity[:])

            statet = wpool.tile([126, 1024], F32R, tag="statet", name="statet")
            nc.gpsimd.dma_start(statet[:], statet_d)
            # fused weight loads: 4 large DMAs instead of 16 (the ~1-2 us
            # per-DMA issue overhead dominated kernel startup)
            wa_t = [wpool.tile([126, 4000], F32R, tag=f"wah{h}", name=f"wah{h}")
                    for h in range(2)]
            wb_t = [wpool.tile([126, 4000], F32R, tag=f"wbh{h}", name=f"wbh{h}")
                    for h in range(2)]
            nc.gpsimd.dma_start(wa_t[0][:], wa_d[0])
            nc.sync.dma_start(wa_t[1][:], wa_d[1])
            nc.gpsimd.dma_start(wb_t[0][:], wb_d[0])
            nc.sync.dma_start(wb_t[1][:], wb_d[1])
            wa = [wa_t[u // 4][:, 1000 * (u % 4):1000 * (u % 4 + 1)] for u in range(8)]
            wb = [wb_t[u // 4][:, 1000 * (u % 4):1000 * (u % 4 + 1)] for u in range(8)]

            hT = [hpool.tile([126, 1024], F32R, tag="hta", name="hta"),
                  hpool.tile([126, 1024], F32R, tag="htb", name="htb")]
            # zero-init (the complementary halves of each K-group must stay 0),
            # then DMA row 125 from the statet image (ones in groups 1 and 3;
            # DVE ops can't start at partition 125, DMA can).
            nc.vector.memzero(hT[0][0:125, :])
            nc.vector.memzero(hT[1][0:125, :])
            nc.gpsimd.dma_start(hT[0][125:126, :], statet_d[125:126, :])
            nc.gpsimd.dma_start(hT[1][125:126, :], statet_d[125:126, :])

            sums = consts.tile([128, 2 * L], F32, tag="sums", name="sums")

            hprev = None  # packed [128, 250] h of the previous step
            hnew = None
            for k in range(1, L + 1):
                first = k == 1
                lhs_tile = statet if first else hT[k % 2]
                W = wa if first else wb

                # separate PSUM tiles per bank so the rz consumers don't
                # wait on the n-block matmuls
                grz = gpsum.tile([128, 512], F32, tag="grz", name="grz")
                gn = gpsum.tile([128, 512], F32, tag="gn", name="gn")
                def mm_half(out_ap, c0, us):
                    # groups 0..3 depend only on the pA copies; 4..7 on pB
                    for u in us:
                        ku = 126 if u in (1, 3) else 125
                        lhsT = lhs_tile[0:ku, 128 * u:128 * u + 128]
                        nc.tensor.matmul(out_ap, lhsT,
                                         W[u][0:ku, c0:c0 + 500],
                                         start=(u == 0), stop=(u == 7))  # noqa

                # region 1 = [gr | gz]: both sigmoids + the whole z-path
                # run while region 2 ([gin | ghn]) is still streaming
                mm_half(grz[:, 0:500], 0, range(8))

                rz = work.tile([128, 2 * HALF], F32, tag="rz", name="rz")
                nc.scalar.activation(rz[:], grz[:, 0:500], AF.Sigmoid)
                r = rz[:, 0:250]
                z = rz[:, 250:500]
                zm1 = work.tile([128, HALF], F32, tag="zm1", name="zm1")
                nc.vector.tensor_scalar_sub(zm1[:], z, 1.0)
                zh = work.tile([128, HALF], F32, tag="zh", name="zh")
                if not first:
                    nc.vector.tensor_mul(zh[:], z, hprev[:])

                mm_half(gn[:, 0:500], 500, range(8))

                # chunked chain (2 x 125 cols) so transpose A can start while
                # chunk 1 is still in the vector/scalar pipes
                rhn = work.tile([128, HALF], F32R, tag="rhn", name="rhn")
                targ = work.tile([128, HALF], F32R, tag="targ", name="targ")
                n = work.tile([128, HALF], F32R, tag="n", name="n")
                t2 = work.tile([128, HALF], F32R, tag="t2", name="t2")
                hnew = work.tile([128, HALF], F32R, tag="hnew", name="hnew")
                if k < L:
                    dst = hT[(k + 1) % 2]
                    dstv = dst[:].rearrange("p (u c) -> p u c", c=128)
                for c in (0, 1):
                    s = slice(125 * c, 125 * (c + 1))
                    acc = sums[:, 2 * (k - 1) + c:2 * (k - 1) + c + 1]
                    nc.vector.tensor_mul(rhn[:, s], r[:, s], gn[:, 250 + 125 * c:250 + 125 * (c + 1)])
                    nc.vector.tensor_add(targ[:, s], rhn[:, s], gn[:, 125 * c:125 * (c + 1)])
                    nc.scalar.activation(n[:, s], targ[:, s], AF.Tanh)
                    if first:
                        # h == 0: h_new = n * (1 - z) = (-n) * (z - 1)
                        nc.vector.scalar_tensor_tensor(
                            hnew[:, s], n[:, s], -1.0, zm1[:, s],
                            op0=ALU.mult, op1=ALU.mult, accum_out=acc)
                    else:
                        nc.vector.scalar_tensor_tensor(
                            t2[:, s], n[:, s], -1.0, zm1[:, s],
                            op0=ALU.mult, op1=ALU.mult)
                        nc.vector.scalar_tensor_tensor(
                            hnew[:, s], t2[:, s], 0.0, zh[:, s],
                            op0=ALU.bypass, op1=ALU.add, accum_out=acc)
                    if k < L:
                        if c == 0:
                            # tiny write-only matmuls on mid-chain tensors:
                            # real PE activity spaced through the tail so the
                            # HAM idle window never completes a full period
                            dmy = tpsum.tile([1, 128], F32, tag="dmy",
                                             name="dmy", bufs=1)
                            for anchor in (rhn, targ, n, t2 if not first else n):
                                nc.tensor.matmul(dmy[:], anchor[0:128, 0:1],
                                                 identity_r[:, 0:128])
                            pA = tpsum.tile([125, 128], F32, tag="pT", name="pA")
                            # transpose via a REGULAR matmul (in.T @ I): unlike
                            # transpose-mode this counts as PE activity for the
                            # HAM clock gate, keeping the array at full clock
                            # through the serial gate-chain tail
                            nc.tensor.matmul(pA[:], hnew[:, 0:125],
                                             identity_r[:])
                            pAv = pA[:].rearrange("p (u c) -> p u c", c=64)
                            # groups 0,1 (G0) on ACT and 2,3 (G1) on DVE run
                            # in parallel -> first 4 next-step matmuls unblock
                            nc.scalar.copy(dstv[0:125, 0:2, 0:64], pAv)
                            nc.vector.tensor_copy(dstv[0:125, 2:4, 64:128], pAv)
                        else:
                            pB = tpsum.tile([125, 128], F32, tag="pT", name="pB")
                            nc.tensor.matmul(pB[:], hnew[:, 125:250],
                                             identity_r[:])
                            pBv = pB[:].rearrange("p (u c) -> p u c", c=64)
                            nc.scalar.copy(dstv[0:125, 4:6, 0:64], pBv)
                            nc.vector.tensor_copy(dstv[0:125, 6:8, 64:128], pBv)
                hprev = hnew

            nc.gpsimd.dma_start(hout_d, hnew[:])
            nc.gpsimd.dma_start(sums_d, sums[:])

    _split_overwide_waits(nc)
    return nc


_NC_CACHE = {}


def _get_nc(L):
    if L not in _NC_CACHE:
        _NC_CACHE[L] = _build(L)
    return _NC_CACHE[L]


def _prep_weights(W_ih, W_hh, b_ih, b_hh):
    """Build wa/wb DRAM images [8, 126, 1000] (grouped, permuted, bias rows)."""
    W_ih = np.asarray(W_ih, np.float32)
    W_hh = np.asarray(W_hh, np.float32)
    b_ih = np.asarray(b_ih, np.float32)
    b_hh = np.asarray(b_hh, np.float32)

    def full(rz_src, n_rows, bias_rz, bias_n):
        rz = rz_src[np.ix_(GATE_ROWS_RZ, PERM)].T          # [500, 1000]
        nn_ = n_rows[:, PERM].T                            # [500, 1000]
        top = np.hstack([rz, nn_])                         # [500, 2000]
        bias = np.hstack([bias_rz, bias_n])[None, :]       # [1, 2000]
        return np.vstack([top, bias]).astype(np.float32)   # [501, 2000]

    zeros = np.zeros((250, D), np.float32)
    bias_rz_sum = (b_ih[:1000] + b_hh[:1000])[GATE_ROWS_RZ]
    bias_n = np.concatenate([b_ih[1000:1250], b_hh[1000:1250],
                             b_ih[1250:1500], b_hh[1250:1500]])

    WB = full(W_ih[:1000] + W_hh[:1000],
              np.vstack([W_ih[1000:1250], W_hh[1000:1250],
                         W_ih[1250:1500], W_hh[1250:1500]]),
              bias_rz_sum, bias_n)
    WA = full(W_ih[:1000],
              np.vstack([W_ih[1000:1250], zeros,
                         W_ih[1250:1500], zeros]),
              bias_rz_sum, bias_n)

    # group u -> (D-block index into the PERM layout, gate-half G)
    DBLK = (0, 1, 0, 1, 2, 3, 2, 3)
    GHALF = (0, 0, 1, 1, 0, 0, 1, 1)

    def pack(Wf):
        out = np.zeros((8, 126, 1000), np.float32)  # regrouped below
        for u in range(8):
            t = DBLK[u]
            lo, hi = (0, 1000) if GHALF[u] == 0 else (500, 1500)
            # region1 = [r | z], region2 = [in | hn]
            rows = Wf[125 * t:125 * (t + 1)]
            out[u, 0:125, 0:500] = rows[:, lo:lo + 500]
            out[u, 0:125, 500:1000] = rows[:, hi:hi + 500]
        # bias row: once per gate-half (groups 1 and 3 have the ones row)
        for u, (lo, hi) in ((1, (0, 1000)), (3, (500, 1500))):
            out[u, 125, 0:500] = Wf[500, lo:lo + 500]
            out[u, 125, 500:1000] = Wf[500, hi:hi + 500]
        return np.ascontiguousarray(
            out.reshape(2, 4, 126, 1000).transpose(0, 2, 1, 3).reshape(2, 126, 4000))

    return pack(WA), pack(WB)


def _prep_state(state):
    """Per-core stationary state^T images [126, 1024]."""
    state = np.asarray(state, np.float32)
    outs = []
    for c in range(NCORES):
        shard = state[BS * c:BS * (c + 1)]            # [64, 500]
        st = shard[:, PERM].T                         # [500, 64]
        img = np.zeros((126, 1024), np.float32)
        DBLK = (0, 1, 0, 1, 2, 3, 2, 3)
        GHALF = (0, 0, 1, 1, 0, 0, 1, 1)
        for u in range(8):
            rows = st[125 * DBLK[u]:125 * (DBLK[u] + 1)]
            off = 128 * u + 64 * GHALF[u]
            img[0:125, off:off + 64] = rows
        img[125, 128 * 1:128 * 1 + 64] = 1.0
        img[125, 128 * 3 + 64:128 * 3 + 128] = 1.0
        outs.append(img)
    return outs


def _run(L, stateTs, wa, wb, trace=False):
    nc = _get_nc(L)
    in_maps = [{"statet": np.ascontiguousarray(stateTs[c]),
                "wa": wa, "wb": wb} for c in range(NCORES)]
    res = bass_utils.run_bass_kernel_spmd(
        nc, in_maps, core_ids=list(range(NCORES)), trace=trace)
    shards = []
    sums = np.zeros((128, 2 * L), np.float64)
    for c in range(NCORES):
        hout = res.results[c]["hout"]
        shards.append(np.concatenate([hout[0:64], hout[64:128]], axis=1))
        sums += res.results[c]["sums"].astype(np.float64)
    h = np.concatenate(shards, axis=0)                # [512, 500]
    means = (sums[:, 0::2] + sums[:, 1::2]).sum(axis=0) / (B * D)  # [L]
    return h, means, res


def kernel(state, W_ih, W_hh, b_ih, b_hh, break_condition, recursion_limit):
    state = np.asarray(state, np.float32)
    L = int(np.asarray(recursion_limit))
    if L <= 0:
        return state.copy()
    bc = float(np.asarray(break_condition))

    wa, wb = _prep_weights(W_ih, W_hh, b_ih, b_hh)
    stateTs = _prep_state(state)

    h, means, _ = _run(L, stateTs, wa, wb)
    fired = np.nonzero(means > bc)[0]
    if fired.size and fired[0] + 1 < L:
        # break fired at step k* = fired[0]+1: output latches h_{k*}
        h, _, _ = _run(int(fired[0]) + 1, stateTs, wa, wb)
    return h.astype(np.float32)



# revision 4
# speedup vs baseline: 1.0447x; 1.0447x over previous
"""Trainium2 Bass kernel for nn_EternalRecursion (GRUCell self-recursion, B=512, D=500).

Strategy
--------
Data-parallel over 8 NeuronCores: 64 batch rows per core, GRU weights replicated.

Math restructuring (host-side):
  - After step 1 the reference feeds h_new as BOTH x and h of the GRU cell, so
    steps >= 2 use combined weights W_rz = (W_ih+W_hh)[0:1000] for the r/z gates,
    while the n-gate keeps W_ih_n / W_hh_n separate (r multiplies only the h-side).
  - Step 1 (x=state, h=0) uses W_ih with a zero block for the h-side n columns:
    same device code path, different weights.
  - Biases fold into the matmuls via an extra contraction row of ones.
  - h_new = n + z*(h-n), so the transposed next-step stationary is computed on
    the PE as T(h_new) = T(n) + T(z*(h-n)) by two accumulating fp16 matmuls,
    keeping the post-matmul serial chain to sigmoid -> mul -> transpose -> copy.

Precision (validated vs the f32 reference: ~1.2e-3 max rel err):
  - r/z gate matmuls run in fp8e4m3 with DoubleRow perf mode (2 K-tiles per
    matmul at 2 rows/cycle). The sigmoid's <=1/4 slope absorbs fp8 noise.
  - n-gate matmuls and the h_new transposes run in fp16.
  - Gate math, carried h state, and the break-check sums stay fp32.

Device layout (per core, per step):
  - h packed [128, 250]: partition 64*H+b holds h[b, 250*H + c].
  - rz PSUM [128, 500] = [r | z], from 4 DoubleRow matmuls (pairs of the 8
    doubled-K groups); n PSUM split per chunk c: [gin_c | ghn_c] [128, 250]
    in a full bank each, from 8 fp16 matmuls per chunk.
  - Software pipeline: matmuls needing transpose-A (h cols 0:125 -> K-groups
    D0/D1) are emitted first each step; chunk-1's transposes + copies of step
    k run during step k+1's prefix, so the serial gate chain of one chunk
    always overlaps the other chunk's / next step's matmul stream.
"""

import os
import sys
import types
import numpy as np
import ml_dtypes

D = 500
B = 512
NCORES = 8
BS = B // NCORES          # 64 batch rows per core
HALF = 250                # free columns of the packed layout
F8NP = ml_dtypes.float8_e4m3fn

# K permutation: hT row-groups are [0:125 | 250:375 | 125:250 | 375:500]
PERM = np.concatenate([
    np.arange(0, 125), np.arange(250, 375),
    np.arange(125, 250), np.arange(375, 500),
])
TBLK = (0, 1, 0, 1, 2, 3, 2, 3)    # n-gate group u -> D-block t
GHALF = (0, 0, 1, 1, 0, 0, 1, 1)   # n-gate group u -> gate half g
PAIR_G = (0, 1, 0, 1)              # rz DoubleRow pair p -> gate half g
PAIR_T = ((0, 1), (0, 1), (2, 3), (2, 3))  # pair p -> (D-block of i=0, i=1)


def _install_hook_module():
    """Provide antenv.axon_hooks (missing from the RO image) so NTFF tracing
    through bass_utils can work when requested. Harmless if anything fails."""
    if "antenv.axon_hooks" in sys.modules:
        return
    mod = types.ModuleType("antenv.axon_hooks")
    holder = [None]
    mod.set_axon_ntff_profile_hook = lambda h: holder.__setitem__(0, h)
    mod.get_axon_ntff_profile_hook = lambda: holder[0]
    sys.modules["antenv.axon_hooks"] = mod
    try:
        from trn_agent_boot.trn_boot import _ntff_profile_via_ctypes
        hook = _ntff_profile_via_ctypes("/opt/axon/libaxon_pjrt.so")
        mod.set_axon_ntff_profile_hook(hook)
    except Exception:
        pass


_install_hook_module()

import concourse.bass as bass  # noqa: E402
import concourse.mybir as mybir  # noqa: E402
import concourse.tile as tile  # noqa: E402
from concourse import bass_utils  # noqa: E402
from concourse.masks import make_identity  # noqa: E402
import bass_rust  # noqa: E402

F32 = mybir.dt.float32
F16 = mybir.dt.float16
F8 = mybir.dt.float8e4
AF = mybir.ActivationFunctionType
ALU = mybir.AluOpType
DR = mybir.MatmulPerfMode.DoubleRow


def _split_overwide_waits(nc, maxw=1):
    """walrus here rejects >1 sync wait per instruction; spread extras over
    preceding NoOp carriers. Most multi-wait instructions get same-engine
    carriers (order-preserving); the kernel-end drain (many loose-end waits)
    gets carriers round-robined across all engines so they resolve in
    parallel before the final barrier instead of serially on one engine."""
    n_new = 0
    all_engines = (mybir.EngineType.SP, mybir.EngineType.Activation,
                   mybir.EngineType.PE, mybir.EngineType.DVE,
                   mybir.EngineType.Pool)
    for fn in nc.m.functions:
        for bb in fn.blocks:
            out = []
            for inst in bb.instructions:
                si = inst.sync_info
                if si is not None and si.on_wait and len(si.on_wait) > maxw:
                    waits = list(si.on_wait)
                    chunks = [waits[i:i + maxw] for i in range(0, len(waits), maxw)]
                    spread = len(chunks) > 4  # only the big end-of-kernel drain
                    for j, ch in enumerate(chunks[:-1]):
                        eng = all_engines[j % len(all_engines)] if spread \
                            else inst.engine
                        nd = mybir.InstNoOp(
                            name=f"I-swx{n_new}", engine=eng,
                            bass_nofuse=True,
                            sync_info=bass_rust.SyncInfo(on_wait=ch, on_update=[]))
                        n_new += 1
                        nc.register_instruction(nd, overwrite=True)
                        out.append(nd)
                    inst.sync_info = bass_rust.SyncInfo(
                        on_wait=chunks[-1], on_update=list(si.on_update or []))
                out.append(inst)
            bb.instructions = out
    return n_new


def _build(L):
    """Build the Bass module for L GRU steps. Returns nc."""
    assert L >= 1
    nc = bass.Bass("TRN2", target_bir_lowering=False, debug=False)

    statet16_d = nc.dram_tensor("statet16", [126, 1024], F16, kind="ExternalInput").ap()
    statet8_d = nc.dram_tensor("statet8", [126, 1024], F8, kind="ExternalInput").ap()
    wn_d = nc.dram_tensor("wn", [2, 126, 4000], F16, kind="ExternalInput").ap()
    wrz_d = nc.dram_tensor("wrz", [2, 126, 4000], F8, kind="ExternalInput").ap()
    hout_d = nc.dram_tensor("hout", [128, HALF], F32, kind="ExternalOutput").ap()
    sums_d = nc.dram_tensor("sums", [128, 2 * L], F32, kind="ExternalOutput").ap()

    with tile.TileContext(nc) as tc:
        import contextlib
        with contextlib.ExitStack() as ctx:
            consts = ctx.enter_context(tc.tile_pool(name="consts", bufs=1))
            wpool = ctx.enter_context(tc.tile_pool(name="weights", bufs=1))
            hpool = ctx.enter_context(tc.tile_pool(name="hstate", bufs=1))
            work = ctx.enter_context(tc.tile_pool(name="work", bufs=2))
            gpsum = ctx.enter_context(tc.tile_pool(name="gpsum", bufs=2, space="PSUM"))
            tpsum = ctx.enter_context(tc.tile_pool(name="tpsum", bufs=1, space="PSUM"))

            identity = consts.tile([128, 128], F32, tag="identity", name="identity")
            make_identity(nc, identity[:])
            ident16 = consts.tile([128, 128], F16, tag="ident16", name="ident16")
            nc.vector.tensor_copy(ident16[:], identity[:])

            statet16 = wpool.tile([126, 1024], F16, tag="statet16", name="statet16")
            statet8 = wpool.tile([126, 1024], F8, tag="statet8", name="statet8")
            nc.gpsimd.dma_start(statet16[:], statet16_d)
            nc.gpsimd.dma_start(statet8[:], statet8_d)
            wn_t = [wpool.tile([126, 4000], F16, tag=f"wn{j}", name=f"wn{j}")
                    for j in range(2)]
            wrz_t = [wpool.tile([126, 4000], F8, tag=f"wrz{j}", name=f"wrz{j}")
                     for j in range(2)]
            nc.sync.dma_start(wrz_t[0][:], wrz_d[0])
            nc.gpsimd.dma_start(wn_t[0][:], wn_d[0])
            nc.sync.dma_start(wrz_t[1][:], wrz_d[1])
            nc.gpsimd.dma_start(wn_t[1][:], wn_d[1])

            hT16 = [hpool.tile([126, 1024], F16, tag=f"ht16{i}", name=f"ht16{i}")
                    for i in range(2)]
            hT8 = [hpool.tile([126, 1024], F8, tag=f"ht8{i}", name=f"ht8{i}")
                   for i in range(2)]
            h32 = [hpool.tile([128, HALF], F32, tag=f"h32{i}", name=f"h32{i}")
                   for i in range(2)]
            # zero the data rows; DMA row 125 (the bias ones live there) from
            # the state images (DVE ops can't start at partition 125, DMA can)
            for i in range(2):
                nc.vector.memzero(hT16[i][0:125, :])
                nc.vector.memzero(hT8[i][0:125, :])
                nc.gpsimd.dma_start(hT16[i][125:126, :], statet16_d[125:126, :])
                nc.gpsimd.dma_start(hT8[i][125:126, :], statet8_d[125:126, :])
            nc.vector.memzero(h32[1][:])

            sums = consts.tile([128, 2 * L], F32, tag="sums", name="sums")

            prev_n16 = None
            prev_zt16 = None
            for k in range(1, L + 1):
                first = k == 1
                rdbuf = k % 2
                dstbuf = (k + 1) % 2
                lhs16 = statet16 if first else hT16[rdbuf]
                lhs8 = statet8 if first else hT8[rdbuf]
                w16 = wn_t[0 if first else 1]
                w8 = wrz_t[0 if first else 1]

                rzp = gpsum.tile([128, 512], F32, tag="rzp", name="rzp")
                gnp = [gpsum.tile([128, 512], F32, tag=f"gnp{c}", name=f"gnp{c}")
                       for c in range(2)]

                def rz_mm(p, start, stop):
                    Kp = 126 if p < 2 else 125
                    lt = lhs8[0:Kp, 256 * p:256 * p + 256].rearrange(
                        "p (i c) -> p i c", c=128)
                    rt = w8[0:Kp, 1000 * p:1000 * p + 1000].rearrange(
                        "p (i n) -> p i n", n=500)
                    nc.tensor.matmul(rzp[:, 0:500], lt, rt, start=start,
                                     stop=stop, perf_mode=DR,
                                     skip_group_check=True)

                def n_mm(u, c, start, stop):
                    Ku = 126 if u in (1, 3) else 125
                    lt = lhs16[0:Ku, 128 * u:128 * u + 128]
                    rt = w16[0:Ku, 500 * u + 250 * c:500 * u + 250 * c + 250]
                    nc.tensor.matmul(gnp[c][:, 0:250], lt, rt, start=start,
                                     stop=stop, skip_group_check=True)

                def t_mm(dst, src_ap, start, stop):
                    nc.tensor.matmul(dst, src_ap, ident16[:], start=start,
                                     stop=stop, skip_group_check=True)

                def copies(tp, half, dst16, dst8):
                    """Copy the [125,128] transpose PSUM into the hT slot
                    halves: half 0 -> groups 0-3 / pairs 0-1 (cols 0:512),
                    half 1 -> groups 4-7 / pairs 2-3 (cols 512:1024)."""
                    tv = tp[:].rearrange("p (u c) -> p u c", c=64)
                    o = 512 * half
                    d0_16 = dst16[0:125, o:o + 256].rearrange(
                        "p (u c) -> p u c", c=128)[:, :, 0:64]
                    d1_16 = dst16[0:125, o + 256:o + 512].rearrange(
                        "p (u c) -> p u c", c=128)[:, :, 64:128]
                    d0_8 = dst8[0:125, o:o + 256].rearrange(
                        "p (i c) -> p i c", c=128)[:, :, 0:64]
                    d1_8 = dst8[0:125, o + 256:o + 512].rearrange(
                        "p (i c) -> p i c", c=128)[:, :, 64:128]
                    nc.scalar.copy(d0_8, tv)
                    nc.vector.tensor_copy(d1_8, tv)
                    nc.scalar.copy(d0_16, tv)
                    nc.vector.tensor_copy(d1_16, tv)

                # ---- prefix: matmuls that only need transpose-A of h_{k-1},
                # plus chunk-1 transposes+copies of step k-1 ----
                if prev_n16 is not None:
                    tpB = tpsum.tile([125, 128], F32, tag="tpB", name="tpB")
                    t_mm(tpB[:], prev_n16[:, 125:250], True, False)
                rz_mm(0, True, False)
                rz_mm(1, False, False)
                if prev_n16 is not None:
                    t_mm(tpB[:], prev_zt16[:, 125:250], False, True)
                    copies(tpB, 1, hT16[rdbuf], hT8[rdbuf])
                for u in (0, 1, 2, 3):
                    n_mm(u, 0, u == 0, False)
                for u in (0, 1, 2, 3):
                    n_mm(u, 1, u == 0, False)

                # ---- suffix: matmuls needing transpose-B ----
                rz_mm(2, False, False)
                rz_mm(3, False, True)
                for u in (4, 5, 6, 7):
                    n_mm(u, 0, False, u == 7)
                for u in (4, 5, 6, 7):
                    n_mm(u, 1, False, u == 7)

                # ---- gate chains ----
                r = work.tile([128, HALF], F32, tag="r", name="r")
                z = work.tile([128, HALF], F32, tag="z", name="z")
                nc.scalar.activation(r[:], rzp[:, 0:250], AF.Sigmoid)
                nc.scalar.activation(z[:], rzp[:, 250:500], AF.Sigmoid)
                rhn = work.tile([128, HALF], F32, tag="rhn", name="rhn")
                targ = work.tile([128, HALF], F32, tag="targ", name="targ")
                hmn = work.tile([128, HALF], F32, tag="hmn", name="hmn")
                n16 = work.tile([128, HALF], F16, tag="n16", name="n16")
                zt16 = work.tile([128, HALF], F16, tag="zt16", name="zt16")
                for c in (0, 1):
                    cs = slice(125 * c, 125 * (c + 1))
                    acc = sums[:, 2 * (k - 1) + c:2 * (k - 1) + c + 1]
                    nc.vector.tensor_mul(rhn[:, cs], r[:, cs], gnp[c][:, 125:250])
                    nc.vector.tensor_add(targ[:, cs], rhn[:, cs], gnp[c][:, 0:125])
                    nc.scalar.activation(n16[:, cs], targ[:, cs], AF.Tanh)
                    nc.vector.tensor_sub(hmn[:, cs], h32[rdbuf][:, cs], n16[:, cs])
                    nc.vector.tensor_mul(zt16[:, cs], z[:, cs], hmn[:, cs])
                    nc.vector.scalar_tensor_tensor(
                        h32[dstbuf][:, cs], n16[:, cs], 0.0, zt16[:, cs],
                        op0=ALU.bypass, op1=ALU.add, accum_out=acc)

                if k < L:
                    tpA = tpsum.tile([125, 128], F32, tag="tpA", name="tpA")
                    t_mm(tpA[:], n16[:, 0:125], True, False)
                    t_mm(tpA[:], zt16[:, 0:125], False, True)
                    copies(tpA, 0, hT16[dstbuf], hT8[dstbuf])
                    prev_n16, prev_zt16 = n16, zt16
                else:
                    prev_n16 = prev_zt16 = None

            nc.gpsimd.dma_start(hout_d, h32[(L + 1) % 2][:])
            nc.gpsimd.dma_start(sums_d, sums[:])

    _split_overwide_waits(nc)
    return nc


_NC_CACHE = {}


def _get_nc(L):
    if L not in _NC_CACHE:
        _NC_CACHE[L] = _build(L)
    return _NC_CACHE[L]


def _prep_weights(W_ih, W_hh, b_ih, b_hh):
    """Build the DRAM weight images: wrz [2, 126, 4000] fp8 (DoubleRow pair
    layout) and wn [2, 126, 4000] fp16 (chunk-interleaved n-gate layout).
    Index 0 = step-1 (x=state, h=0) weights, 1 = steady-state weights."""
    W_ih = np.asarray(W_ih, np.float32)
    W_hh = np.asarray(W_hh, np.float32)
    b_ih = np.asarray(b_ih, np.float32)
    b_hh = np.asarray(b_hh, np.float32)

    def rz_img(Wrz, brz):
        img = np.zeros((126, 4000), np.float32)
        for p in range(4):
            g = PAIR_G[p]
            rows = np.concatenate([np.arange(250 * g, 250 * g + 250),
                                   np.arange(500 + 250 * g, 500 + 250 * g + 250)])
            for i, t in enumerate(PAIR_T[p]):
                cols = PERM[125 * t:125 * (t + 1)]
                img[0:125, 1000 * p + 500 * i:1000 * p + 500 * i + 500] = \
                    Wrz[np.ix_(rows, cols)].T
            if p < 2:
                img[125, 1000 * p + 500:1000 * p + 1000] = brz[rows]
        return img.astype(F8NP)

    def n_img(Win, Whn, bin_, bhn):
        img = np.zeros((126, 4000), np.float32)
        for u in range(8):
            t, g = TBLK[u], GHALF[u]
            cols = PERM[125 * t:125 * (t + 1)]
            base = 500 * u
            for c in range(2):
                ch = np.arange(250 * g + 125 * c, 250 * g + 125 * c + 125)
                img[0:125, base + 250 * c:base + 250 * c + 125] = \
                    Win[np.ix_(ch, cols)].T
                img[0:125, base + 250 * c + 125:base + 250 * c + 250] = \
                    Whn[np.ix_(ch, cols)].T
                if u in (1, 3):
                    img[125, base + 250 * c:base + 250 * c + 125] = bin_[ch]
                    img[125, base + 250 * c + 125:base + 250 * c + 250] = bhn[ch]
        return img.astype(np.float16)

    Win = W_ih[1000:1500]
    Whn = W_hh[1000:1500]
    zeros_w = np.zeros_like(Whn)
    zeros_b = np.zeros(500, np.float32)
    wrz = np.stack([rz_img(W_ih[:1000], b_ih[:1000]),
                    rz_img(W_ih[:1000] + W_hh[:1000], b_ih[:1000] + b_hh[:1000])])
    wn = np.stack([n_img(Win, zeros_w, b_ih[1000:1500], zeros_b),
                   n_img(Win, Whn, b_ih[1000:1500], b_hh[1000:1500])])
    return np.ascontiguousarray(wrz), np.ascontiguousarray(wn)


def _prep_state(state):
    """Per-core stationary state^T images: fp16 [126, 1024] (group-major) and
    fp8 [126, 1024] (DoubleRow pair-major)."""
    state = np.asarray(state, np.float32)
    outs = []
    for cidx in range(NCORES):
        shard = state[BS * cidx:BS * (cidx + 1)]      # [64, 500]
        st = shard[:, PERM].T                          # [500, 64]
        s16 = np.zeros((126, 1024), np.float32)
        for u in range(8):
            t, g = TBLK[u], GHALF[u]
            s16[0:125, 128 * u + 64 * g:128 * u + 64 * g + 64] = \
                st[125 * t:125 * (t + 1)]
        s16[125, 128 * 1:128 * 1 + 64] = 1.0
        s16[125, 128 * 3 + 64:128 * 3 + 128] = 1.0
        s8 = np.zeros((126, 1024), np.float32)
        for p in range(4):
            g = PAIR_G[p]
            for i, t in enumerate(PAIR_T[p]):
                off = 256 * p + 128 * i + 64 * g
                s8[0:125, off:off + 64] = st[125 * t:125 * (t + 1)]
            if p < 2:
                s8[125, 256 * p + 128 + 64 * g:256 * p + 128 + 64 * g + 64] = 1.0
        outs.append((s16.astype(np.float16), s8.astype(F8NP)))
    return outs


def _run(L, stateTs, wrz, wn, trace=False):
    nc = _get_nc(L)
    in_maps = [{"statet16": np.ascontiguousarray(stateTs[c][0]),
                "statet8": np.ascontiguousarray(stateTs[c][1]),
                "wrz": wrz, "wn": wn} for c in range(NCORES)]
    res = bass_utils.run_bass_kernel_spmd(
        nc, in_maps, core_ids=list(range(NCORES)), trace=trace)
    shards = []
    sums = np.zeros((128, 2 * L), np.float64)
    for c in range(NCORES):
        hout = res.results[c]["hout"]
        shards.append(np.concatenate([hout[0:64], hout[64:128]], axis=1))
        sums += res.results[c]["sums"].astype(np.float64)
    h = np.concatenate(shards, axis=0)                # [512, 500]
    means = (sums[:, 0::2] + sums[:, 1::2]).sum(axis=0) / (B * D)  # [L]
    return h, means, res


def kernel(state, W_ih, W_hh, b_ih, b_hh, break_condition, recursion_limit):
    state = np.asarray(state, np.float32)
    L = int(np.asarray(recursion_limit))
    if L <= 0:
        return state.copy()
    bc = float(np.asarray(break_condition))

    wrz, wn = _prep_weights(W_ih, W_hh, b_ih, b_hh)
    stateTs = _prep_state(state)

    h, means, _ = _run(L, stateTs, wrz, wn)
    fired = np.nonzero(means > bc)[0]
    if fired.size and fired[0] + 1 < L:
        # break fired at step k* = fired[0]+1: output latches h_{k*}
        h, _, _ = _run(int(fired[0]) + 1, stateTs, wrz, wn)
    return h.astype(np.float32)
